# revision 16
# baseline (speedup 1.0000x reference)
"""Trainium2 Bass kernel for a LLaMA-style causal attention block.

Sharding (8 NeuronCores, one trn2 chip):
  - Tensor-parallel over heads: core c owns heads [4c, 4c+4) -> wq/wk/wv column
    slices [4096, 512]; computes qT/kT/v + RoPE + causal attention for its heads.
  - attnT [512, 2048] (bf16) is AllGather'd (chunked over 4 sq quarters, so comm
    overlaps compute) -> each core computes out[:, 512c:512c+512] = attn @ wo_cols.
  - Host concatenates the 8 column slices.

Layout trick: everything is computed transposed ([head_dim, seq]) so that no
on-device transposes are needed anywhere:
  qT/kT = w_h.T @ xT      (xT host-pretransposed)
  scoresT[sk, sq] = kT_tile.T @ qT     (softmax denom on DVE/GpSimd, not PE)
  attnT[hd, sq] = v_tile.T @ expT      (expT is exactly the scoresT layout)
  out[sq, cols] = attnT_full_tile.T @ wo_tile
RoPE is applied in the transposed layout with a DVE stream_shuffle partition
pair-swap. exp() needs no max-subtraction: scores are O(1) by construction.

Compute dtype bf16 (f32 PSUM accumulation), I/O f32.
"""

import math
import os
import sys

for _p in ("/opt/trn_rl_repo",):
    if os.path.isdir(_p) and _p not in sys.path:
        sys.path.insert(0, _p)

import numpy as np
import ml_dtypes

N_CORES = 8
B, S, D, H = 1, 2048, 4096, 32
HD = D // H          # 128
HPC = H // N_CORES   # 4 heads per core
CW = D // N_CORES    # 512 columns per core
NK = D // 128        # 32 contraction tiles
SQT = 512            # sq tile width
NSQ = S // SQT       # 4
SCALE = 1.0 / math.sqrt(HD)

_CACHE = {}
LAST_RESULT = None   # test harness reads exec_time_ns from here


def _build():
    import concourse.mybir as mybir
    import concourse.tile as tile
    from concourse import bacc, bass_isa

    dt = mybir.dt
    f32, bf16 = dt.float32, dt.bfloat16

    nc = bacc.Bacc("TRN2", target_bir_lowering=False, debug=False,
                   num_devices=N_CORES)

    xT = nc.dram_tensor("xT", [D, S], bf16, kind="ExternalInput").ap()
    wq = nc.dram_tensor("wq", [D, CW], bf16, kind="ExternalInput").ap()
    wk = nc.dram_tensor("wk", [D, CW], bf16, kind="ExternalInput").ap()
    wv = nc.dram_tensor("wv", [D, CW], bf16, kind="ExternalInput").ap()
    wo = nc.dram_tensor("wo", [D, CW], bf16, kind="ExternalInput").ap()
    cosT = nc.dram_tensor("cosT", [HD, S], bf16, kind="ExternalInput").ap()
    sinT = nc.dram_tensor("sinT", [HD, S], bf16, kind="ExternalInput").ap()
    masks = nc.dram_tensor("masks", [4, 128, SQT], bf16, kind="ExternalInput").ap()
    out = nc.dram_tensor("out", [S, CW], f32, kind="ExternalOutput").ap()

    swap_mask = []
    for i in range(16):
        swap_mask += [2 * i + 1, 2 * i]

    rg = [list(range(N_CORES))]

    with tile.TileContext(nc) as tc:
        with (
            tc.tile_pool(name="consts", bufs=1) as cpool,
            tc.tile_pool(name="xp", bufs=34) as xpool,
            tc.tile_pool(name="wqp", bufs=6) as wqp,
            tc.tile_pool(name="wkp", bufs=6) as wkp,
            tc.tile_pool(name="wvp", bufs=6) as wvp,
            tc.tile_pool(name="res", bufs=1) as res,
            tc.tile_pool(name="rope32", bufs=3) as rope32,
            tc.tile_pool(name="ropebf", bufs=4) as ropebf,
            tc.tile_pool(name="expp", bufs=10) as expp,
            tc.tile_pool(name="nrm", bufs=2) as nrm,
            tc.tile_pool(name="attnsb", bufs=2) as attnsb,
            tc.tile_pool(name="wop", bufs=8) as wop,
            tc.tile_pool(name="agsb", bufs=8) as agsb,
            tc.tile_pool(name="osb", bufs=2) as osb,
            tc.tile_pool(name="ps", bufs=8, space="PSUM") as ps,
            tc.tile_pool(name="dram", bufs=1, space="DRAM") as dram,
        ):
            # resident results of QKV+rope
            qrot = [res.tile([HD, S], bf16, name=f"qrot{h}") for h in range(HPC)]
            krot = [res.tile([HD, S], bf16, name=f"krot{h}") for h in range(HPC)]
            v_sb = [res.tile([128, CW], bf16, name=f"v{i}") for i in range(S // 128)]

            # AllGather bounce buffers (one per sq quarter)
            ag_in = [dram.tile([HPC * HD, SQT], bf16, name=f"agin{q}")
                     for q in range(NSQ)]
            ag_out = [dram.tile([D, SQT], bf16, addr_space="Shared",
                                name=f"agout{q}") for q in range(NSQ)]

            cos_sb = cpool.tile([HD, S], bf16, name="cos_sb")
            sin_sb = cpool.tile([HD, S], bf16, name="sin_sb")
            mask_sb = [cpool.tile([128, SQT], bf16, name=f"mask{r}")
                       for r in range(4)]

            def emit_qkv(st):
                sq0 = st * SQT
                q_ps = [ps.tile([128, SQT], f32, tag="b", name=f"qps{st}_{h}")
                        for h in range(HPC)]
                k_ps = [ps.tile([128, SQT], f32, tag="b", name=f"kps{st}_{h}")
                        for h in range(HPC)]
                x_tiles = []
                for d in range(NK):
                    xt = xpool.tile([128, SQT], bf16, tag="x", name=f"x{st}_{d}")
                    nc.sync.dma_start(xt[:], xT[d * 128:(d + 1) * 128,
                                                sq0:sq0 + SQT])
                    x_tiles.append(xt)
                    wqt = wqp.tile([128, CW], bf16, tag="wq", name=f"wq{st}_{d}")
                    nc.sync.dma_start(wqt[:], wq[d * 128:(d + 1) * 128, :])
                    wkt = wkp.tile([128, CW], bf16, tag="wk", name=f"wk{st}_{d}")
                    nc.sync.dma_start(wkt[:], wk[d * 128:(d + 1) * 128, :])
                    first, last = d == 0, d == NK - 1
                    for h in range(HPC):
                        nc.tensor.matmul(q_ps[h][:], wqt[:, h * HD:(h + 1) * HD],
                                         xt[:], start=first, stop=last)
                    for h in range(HPC):
                        nc.tensor.matmul(k_ps[h][:], wkt[:, h * HD:(h + 1) * HD],
                                         xt[:], start=first, stop=last)
                if st == 0:
                    # constants are first needed by RoPE / attention below;
                    # emitting them here keeps the first QKV DMAs in front
                    nc.sync.dma_start(cos_sb[:], cosT[:])
                    nc.sync.dma_start(sin_sb[:], sinT[:])
                    for r in range(4):
                        nc.sync.dma_start(mask_sb[r][:], masks[r])
                # RoPE: rot = t*cos + shuffle(t)*sin'   (sin' sign-baked)
                for h in range(HPC):
                    for pst, rot in ((q_ps[h], qrot[h]), (k_ps[h], krot[h])):
                        tbf = ropebf.tile([128, SQT], bf16, tag="rbf",
                                          name=f"rbf{st}_{h}")
                        nc.scalar.copy(tbf[:], pst[:])
                        tsw = ropebf.tile([128, SQT], bf16, tag="rsw",
                                          name=f"rsw{st}_{h}")
                        nc.vector.stream_shuffle(tsw[:], tbf[:], swap_mask)
                        t1 = rope32.tile([128, SQT], f32, tag="r32",
                                         name=f"r1_{st}_{h}")
                        nc.vector.tensor_mul(t1[:], tbf[:],
                                             cos_sb[:, sq0:sq0 + SQT])
                        t2 = rope32.tile([128, SQT], f32, tag="r32",
                                         name=f"r2_{st}_{h}")
                        nc.vector.tensor_mul(t2[:], tsw[:],
                                             sin_sb[:, sq0:sq0 + SQT])
                        nc.vector.tensor_add(rot[:, sq0:sq0 + SQT], t1[:], t2[:])
                # V projection for this s range
                v_ps = [ps.tile([128, CW], f32, tag="b", name=f"vps{st}_{ss}")
                        for ss in range(4)]
                for d in range(NK):
                    wvt = wvp.tile([128, CW], bf16, tag="wv", name=f"wv{st}_{d}")
                    nc.sync.dma_start(wvt[:], wv[d * 128:(d + 1) * 128, :])
                    first, last = d == 0, d == NK - 1
                    for ss in range(4):
                        nc.tensor.matmul(v_ps[ss][:],
                                         x_tiles[d][:, ss * 128:(ss + 1) * 128],
                                         wvt[:], start=first, stop=last)
                for ss in range(4):
                    nc.scalar.copy(v_sb[st * 4 + ss][:], v_ps[ss][:])

            def emit_attention(sqT):
                sq0 = sqT * SQT
                nblk = 4 * (sqT + 1)
                for h in range(HPC):
                    attn_ps = ps.tile([HD, SQT], f32, tag="b",
                                      name=f"aps{sqT}_{h}")
                    acc = nrm.tile([128, SQT], f32, tag="acc",
                                   name=f"acc{sqT}_{h}")
                    exp_tiles = []

                    def emit_pv(j, h=h, attn_ps=attn_ps,
                                exp_tiles=exp_tiles, nblk=nblk):
                        nc.tensor.matmul(attn_ps[:],
                                         v_sb[j][:, h * HD:(h + 1) * HD],
                                         exp_tiles[j][:],
                                         start=(j == 0), stop=(j == nblk - 1))

                    for i in range(nblk):
                        sc = ps.tile([128, SQT], f32, tag="b",
                                     name=f"sc{sqT}_{h}_{i}")
                        nc.tensor.matmul(sc[:],
                                         krot[h][:, i * 128:(i + 1) * 128],
                                         qrot[h][:, sq0:sq0 + SQT],
                                         start=True, stop=True)
                        r = i - 4 * sqT
                        if r >= 0:  # block straddles the diagonal
                            nc.vector.tensor_add(sc[:], sc[:], mask_sb[r][:])
                        e = expp.tile([128, SQT], bf16, tag="e",
                                      name=f"e{sqT}_{h}_{i}")
                        nc.scalar.activation(e[:], sc[:],
                                             mybir.ActivationFunctionType.Exp,
                                             scale=SCALE)
                        exp_tiles.append(e)
                        # softmax denominator accumulates on DVE; PE stays free
                        if i == 0:
                            nc.vector.tensor_copy(acc[:], e[:])
                        else:
                            nc.vector.tensor_add(acc[:], acc[:], e[:])
                        if i >= 2:
                            emit_pv(i - 2)
                    emit_pv(nblk - 2)
                    emit_pv(nblk - 1)

                    # evacuate PSUM right away so the next round's projections
                    # get their banks back without waiting on the normalize
                    a32 = nrm.tile([HD, SQT], f32, tag="a32",
                                   name=f"a32_{sqT}_{h}")
                    nc.scalar.copy(a32[:], attn_ps[:])
                    red = nrm.tile([128, SQT], f32, tag="red",
                                   name=f"red{sqT}_{h}")
                    nc.gpsimd.partition_all_reduce(
                        red[:], acc[:], channels=128,
                        reduce_op=bass_isa.ReduceOp.add)
                    rec = nrm.tile([128, SQT], f32, tag="rec",
                                   name=f"rec{sqT}_{h}")
                    nc.vector.reciprocal(rec[:], red[:])
                    a_sb = attnsb.tile([HD, SQT], bf16, tag="a",
                                       name=f"asb{sqT}_{h}")
                    nc.vector.tensor_mul(a_sb[:], a32[:], rec[:])
                    nc.sync.dma_start(ag_in[sqT][h * HD:(h + 1) * HD, :],
                                      a_sb[:])
                # gather this sq quarter's attnT from all cores
                nc.gpsimd.collective_compute(
                    "AllGather", mybir.AluOpType.bypass, replica_groups=rg,
                    ins=[ag_in[sqT].opt()], outs=[ag_out[sqT].opt()])

            for st in range(NSQ):
                emit_qkv(st)
                emit_attention(st)

            # ================= output projection =================
            for q in range(NSQ):
                o_ps = [ps.tile([128, CW], f32, tag="b", name=f"ops{q}_{ss}")
                        for ss in range(4)]
                for d in range(NK):
                    agt = agsb.tile([128, SQT], bf16, tag="ag",
                                    name=f"agt{q}_{d}")
                    nc.sync.dma_start(agt[:],
                                      ag_out[q][d * 128:(d + 1) * 128, :])
                    wot = wop.tile([128, CW], bf16, tag="wo",
                                   name=f"wot{q}_{d}")
                    nc.sync.dma_start(wot[:], wo[d * 128:(d + 1) * 128, :])
                    first, last = d == 0, d == NK - 1
                    for ss in range(4):
                        nc.tensor.matmul(o_ps[ss][:],
                                         agt[:, ss * 128:(ss + 1) * 128],
                                         wot[:], start=first, stop=last)
                for ss in range(4):
                    o = osb.tile([128, CW], f32, tag="o", name=f"o{q}_{ss}")
                    nc.scalar.copy(o[:], o_ps[ss][:])
                    nc.sync.dma_start(
                        out[q * SQT + ss * 128:q * SQT + (ss + 1) * 128, :],
                        o[:])

    nc.compile()
    return nc


def _prep_inputs(x, wq, wk, wv, wo, freqs_cos, freqs_sin, mask):
    bf16 = ml_dtypes.bfloat16
    x2 = np.asarray(x, dtype=np.float32).reshape(S, D)
    xT = np.ascontiguousarray(x2.T).astype(bf16)
    cosT = np.repeat(np.asarray(freqs_cos, np.float32).T, 2, axis=0)
    sinT = np.repeat(np.asarray(freqs_sin, np.float32).T, 2, axis=0).copy()
    sinT[0::2] *= -1.0
    cosT = np.ascontiguousarray(cosT).astype(bf16)
    sinT = np.ascontiguousarray(sinT).astype(bf16)
    m2 = np.asarray(mask, np.float32).reshape(S, S)
    masks = np.stack([np.ascontiguousarray(m2[0:SQT, r * 128:(r + 1) * 128].T)
                      for r in range(4)]).astype(bf16)  # [4, 128, 512]
    in_maps = []
    for c in range(N_CORES):
        cols = slice(c * CW, (c + 1) * CW)
        in_maps.append({
            "xT": xT,
            "wq": np.ascontiguousarray(np.asarray(wq, np.float32)[:, cols]).astype(bf16),
            "wk": np.ascontiguousarray(np.asarray(wk, np.float32)[:, cols]).astype(bf16),
            "wv": np.ascontiguousarray(np.asarray(wv, np.float32)[:, cols]).astype(bf16),
            "wo": np.ascontiguousarray(np.asarray(wo, np.float32)[:, cols]).astype(bf16),
            "cosT": cosT,
            "sinT": sinT,
            "masks": masks,
        })
    return in_maps


def kernel(x, wq, wk, wv, wo, freqs_cos, freqs_sin, mask):
    global LAST_RESULT
    from concourse.bass_utils import run_bass_kernel_spmd

    if "nc" not in _CACHE:
        _CACHE["nc"] = _build()
    nc = _CACHE["nc"]
    in_maps = _prep_inputs(x, wq, wk, wv, wo, freqs_cos, freqs_sin, mask)
    res = run_bass_kernel_spmd(nc, in_maps, core_ids=list(range(N_CORES)))
    LAST_RESULT = res
    out = np.concatenate([res.results[c]["out"] for c in range(N_CORES)],
                         axis=1)
    return out.reshape(B, S, D).astype(np.float32)


# revision 17
# speedup vs baseline: 1.0491x; 1.0491x over previous
"""Trainium2 Bass kernel for a LLaMA-style causal attention block.

Sharding (8 NeuronCores, one trn2 chip):
  - Tensor-parallel over heads: core c owns heads [4c, 4c+4) -> wq/wk/wv column
    slices [4096, 512]; computes qT/kT/v + RoPE + causal attention for its heads.
  - attnT [512, 2048] (bf16) is AllGather'd (chunked over 4 sq quarters, so comm
    overlaps compute) -> each core computes out[:, 512c:512c+512] = attn @ wo_cols.
  - Host concatenates the 8 column slices.

Layout trick: everything is computed transposed ([head_dim, seq]) so that no
on-device transposes are needed anywhere:
  qT/kT = w_h.T @ xT      (xT host-pretransposed)
  scoresT[sk, sq] = kT_tile.T @ qT     (softmax denom on DVE/GpSimd, not PE)
  attnT[hd, sq] = v_tile.T @ expT      (expT is exactly the scoresT layout)
  out[sq, cols] = attnT_full_tile.T @ wo_tile
RoPE is applied in the transposed layout with a DVE stream_shuffle partition
pair-swap. exp() needs no max-subtraction: scores are O(1) by construction.

Compute dtype bf16 (f32 PSUM accumulation), I/O f32.
"""

import math
import os
import sys

for _p in ("/opt/trn_rl_repo",):
    if os.path.isdir(_p) and _p not in sys.path:
        sys.path.insert(0, _p)

import numpy as np
import ml_dtypes

N_CORES = 8
B, S, D, H = 1, 2048, 4096, 32
HD = D // H          # 128
HPC = H // N_CORES   # 4 heads per core
CW = D // N_CORES    # 512 columns per core
NK = D // 128        # 32 contraction tiles
SQT = 512            # sq tile width
NSQ = S // SQT       # 4
SCALE = 1.0 / math.sqrt(HD)

_CACHE = {}
LAST_RESULT = None   # test harness reads exec_time_ns from here


def _build():
    import concourse.mybir as mybir
    import concourse.tile as tile
    from concourse import bacc, bass_isa

    dt = mybir.dt
    f32, bf16 = dt.float32, dt.bfloat16

    nc = bacc.Bacc("TRN2", target_bir_lowering=False, debug=False,
                   num_devices=N_CORES)

    xT = nc.dram_tensor("xT", [D, S], bf16, kind="ExternalInput").ap()
    wq = nc.dram_tensor("wq", [D, CW], bf16, kind="ExternalInput").ap()
    wk = nc.dram_tensor("wk", [D, CW], bf16, kind="ExternalInput").ap()
    wv = nc.dram_tensor("wv", [D, CW], bf16, kind="ExternalInput").ap()
    wo = nc.dram_tensor("wo", [D, CW], bf16, kind="ExternalInput").ap()
    cosT = nc.dram_tensor("cosT", [HD, S], bf16, kind="ExternalInput").ap()
    sinT = nc.dram_tensor("sinT", [HD, S], bf16, kind="ExternalInput").ap()
    ones = nc.dram_tensor("ones", [HD, 1], bf16, kind="ExternalInput").ap()
    masks = nc.dram_tensor("masks", [4, 128, SQT], bf16, kind="ExternalInput").ap()
    out = nc.dram_tensor("out", [S, CW], f32, kind="ExternalOutput").ap()

    swap_mask = []
    for i in range(16):
        swap_mask += [2 * i + 1, 2 * i]

    rg = [list(range(N_CORES))]

    with tile.TileContext(nc) as tc:
        with (
            tc.tile_pool(name="consts", bufs=1) as cpool,
            tc.tile_pool(name="xp", bufs=34) as xpool,
            tc.tile_pool(name="wqp", bufs=6) as wqp,
            tc.tile_pool(name="wkp", bufs=6) as wkp,
            tc.tile_pool(name="wvp", bufs=6) as wvp,
            tc.tile_pool(name="res", bufs=1) as res,
            tc.tile_pool(name="rope32", bufs=3) as rope32,
            tc.tile_pool(name="ropebf", bufs=4) as ropebf,
            tc.tile_pool(name="expp", bufs=10) as expp,
            tc.tile_pool(name="nrm", bufs=2) as nrm,
            tc.tile_pool(name="attnsb", bufs=2) as attnsb,
            tc.tile_pool(name="wop", bufs=8) as wop,
            tc.tile_pool(name="agsb", bufs=8) as agsb,
            tc.tile_pool(name="osb", bufs=2) as osb,
            tc.tile_pool(name="ps", bufs=8, space="PSUM") as ps,
            tc.tile_pool(name="dram", bufs=1, space="DRAM") as dram,
        ):
            # resident results of QKV+rope
            qrot = [res.tile([HD, S], bf16, name=f"qrot{h}") for h in range(HPC)]
            krot = [res.tile([HD, S], bf16, name=f"krot{h}") for h in range(HPC)]
            v_sb = [res.tile([128, CW], bf16, name=f"v{i}") for i in range(S // 128)]

            # AllGather bounce buffers (one per sq quarter)
            ag_in = [dram.tile([HPC * HD, SQT], bf16, name=f"agin{q}")
                     for q in range(NSQ)]
            ag_out = [dram.tile([D, SQT], bf16, addr_space="Shared",
                                name=f"agout{q}") for q in range(NSQ)]

            cos_sb = cpool.tile([HD, S], bf16, name="cos_sb")
            ones_sb = cpool.tile([HD, 1], bf16, name="ones_sb")
            sin_sb = cpool.tile([HD, S], bf16, name="sin_sb")
            mask_sb = [cpool.tile([128, SQT], bf16, name=f"mask{r}")
                       for r in range(4)]

            def emit_qkv(st):
                sq0 = st * SQT
                q_ps = [ps.tile([128, SQT], f32, tag="b", name=f"qps{st}_{h}")
                        for h in range(HPC)]
                k_ps = [ps.tile([128, SQT], f32, tag="b", name=f"kps{st}_{h}")
                        for h in range(HPC)]
                x_tiles = []
                for d in range(NK):
                    xt = xpool.tile([128, SQT], bf16, tag="x", name=f"x{st}_{d}")
                    nc.sync.dma_start(xt[:], xT[d * 128:(d + 1) * 128,
                                                sq0:sq0 + SQT])
                    x_tiles.append(xt)
                    wqt = wqp.tile([128, CW], bf16, tag="wq", name=f"wq{st}_{d}")
                    nc.sync.dma_start(wqt[:], wq[d * 128:(d + 1) * 128, :])
                    wkt = wkp.tile([128, CW], bf16, tag="wk", name=f"wk{st}_{d}")
                    nc.sync.dma_start(wkt[:], wk[d * 128:(d + 1) * 128, :])
                    first, last = d == 0, d == NK - 1
                    for h in range(HPC):
                        nc.tensor.matmul(q_ps[h][:], wqt[:, h * HD:(h + 1) * HD],
                                         xt[:], start=first, stop=last)
                    for h in range(HPC):
                        nc.tensor.matmul(k_ps[h][:], wkt[:, h * HD:(h + 1) * HD],
                                         xt[:], start=first, stop=last)
                if st == 0:
                    # constants are first needed by RoPE / attention below;
                    # emitting them here keeps the first QKV DMAs in front
                    nc.sync.dma_start(cos_sb[:], cosT[:])
                    nc.sync.dma_start(sin_sb[:], sinT[:])
                    nc.sync.dma_start(ones_sb[:], ones[:])
                    for r in range(4):
                        nc.sync.dma_start(mask_sb[r][:], masks[r])
                # RoPE: rot = t*cos + shuffle(t)*sin'   (sin' sign-baked)
                for h in range(HPC):
                    for pst, rot in ((q_ps[h], qrot[h]), (k_ps[h], krot[h])):
                        tbf = ropebf.tile([128, SQT], bf16, tag="rbf",
                                          name=f"rbf{st}_{h}")
                        nc.scalar.copy(tbf[:], pst[:])
                        tsw = ropebf.tile([128, SQT], bf16, tag="rsw",
                                          name=f"rsw{st}_{h}")
                        nc.vector.stream_shuffle(tsw[:], tbf[:], swap_mask)
                        t1 = rope32.tile([128, SQT], f32, tag="r32",
                                         name=f"r1_{st}_{h}")
                        nc.vector.tensor_mul(t1[:], tbf[:],
                                             cos_sb[:, sq0:sq0 + SQT])
                        t2 = rope32.tile([128, SQT], f32, tag="r32",
                                         name=f"r2_{st}_{h}")
                        nc.vector.tensor_mul(t2[:], tsw[:],
                                             sin_sb[:, sq0:sq0 + SQT])
                        nc.vector.tensor_add(rot[:, sq0:sq0 + SQT], t1[:], t2[:])
                # V projection for this s range
                v_ps = [ps.tile([128, CW], f32, tag="b", name=f"vps{st}_{ss}")
                        for ss in range(4)]
                for d in range(NK):
                    wvt = wvp.tile([128, CW], bf16, tag="wv", name=f"wv{st}_{d}")
                    nc.sync.dma_start(wvt[:], wv[d * 128:(d + 1) * 128, :])
                    first, last = d == 0, d == NK - 1
                    for ss in range(4):
                        nc.tensor.matmul(v_ps[ss][:],
                                         x_tiles[d][:, ss * 128:(ss + 1) * 128],
                                         wvt[:], start=first, stop=last)
                for ss in range(4):
                    nc.scalar.copy(v_sb[st * 4 + ss][:], v_ps[ss][:])

            def emit_attention(sqT):
                sq0 = sqT * SQT
                nblk = 4 * (sqT + 1)
                for h in range(HPC):
                    attn_ps = ps.tile([HD, SQT], f32, tag="b",
                                      name=f"aps{sqT}_{h}")
                    den_ps = ps.tile([1, SQT], f32, tag="b",
                                     name=f"dps{sqT}_{h}")
                    exp_tiles = []

                    def emit_pv(j, h=h, attn_ps=attn_ps, den_ps=den_ps,
                                exp_tiles=exp_tiles, nblk=nblk):
                        first, last = j == 0, j == nblk - 1
                        nc.tensor.matmul(attn_ps[:],
                                         v_sb[j][:, h * HD:(h + 1) * HD],
                                         exp_tiles[j][:],
                                         start=first, stop=last)
                        nc.tensor.matmul(den_ps[:], ones_sb[:],
                                         exp_tiles[j][:],
                                         start=first, stop=last)

                    for i in range(nblk):
                        sc = ps.tile([128, SQT], f32, tag="b",
                                     name=f"sc{sqT}_{h}_{i}")
                        nc.tensor.matmul(sc[:],
                                         krot[h][:, i * 128:(i + 1) * 128],
                                         qrot[h][:, sq0:sq0 + SQT],
                                         start=True, stop=True)
                        r = i - 4 * sqT
                        if r >= 0:  # block straddles the diagonal
                            nc.vector.tensor_add(sc[:], sc[:], mask_sb[r][:])
                        e = expp.tile([128, SQT], bf16, tag="e",
                                      name=f"e{sqT}_{h}_{i}")
                        nc.scalar.activation(e[:], sc[:],
                                             mybir.ActivationFunctionType.Exp,
                                             scale=SCALE)
                        exp_tiles.append(e)
                        if i >= 2:
                            emit_pv(i - 2)
                    emit_pv(nblk - 2)
                    emit_pv(nblk - 1)

                    # evacuate PSUM right away so the next round's projections
                    # get their banks back without waiting on the normalize
                    a32 = nrm.tile([HD, SQT], f32, tag="a32",
                                   name=f"a32_{sqT}_{h}")
                    nc.scalar.copy(a32[:], attn_ps[:])
                    rec = nrm.tile([1, SQT], f32, tag="rec",
                                   name=f"rec{sqT}_{h}")
                    nc.vector.reciprocal(rec[:], den_ps[:])
                    bc = nrm.tile([128, SQT], f32, tag="bc",
                                  name=f"bc{sqT}_{h}")
                    nc.gpsimd.partition_broadcast(bc[:], rec[:], channels=128)
                    a_sb = attnsb.tile([HD, SQT], bf16, tag="a",
                                       name=f"asb{sqT}_{h}")
                    nc.vector.tensor_mul(a_sb[:], a32[:], bc[:])
                    nc.sync.dma_start(ag_in[sqT][h * HD:(h + 1) * HD, :],
                                      a_sb[:])
                # gather this sq quarter's attnT from all cores
                nc.gpsimd.collective_compute(
                    "AllGather", mybir.AluOpType.bypass, replica_groups=rg,
                    ins=[ag_in[sqT].opt()], outs=[ag_out[sqT].opt()])

            for st in range(NSQ):
                emit_qkv(st)
            for st in range(NSQ):
                emit_attention(st)

            # ================= output projection =================
            for q in range(NSQ):
                o_ps = [ps.tile([128, CW], f32, tag="b", name=f"ops{q}_{ss}")
                        for ss in range(4)]
                for d in range(NK):
                    agt = agsb.tile([128, SQT], bf16, tag="ag",
                                    name=f"agt{q}_{d}")
                    nc.sync.dma_start(agt[:],
                                      ag_out[q][d * 128:(d + 1) * 128, :])
                    wot = wop.tile([128, CW], bf16, tag="wo",
                                   name=f"wot{q}_{d}")
                    nc.sync.dma_start(wot[:], wo[d * 128:(d + 1) * 128, :])
                    first, last = d == 0, d == NK - 1
                    for ss in range(4):
                        nc.tensor.matmul(o_ps[ss][:],
                                         agt[:, ss * 128:(ss + 1) * 128],
                                         wot[:], start=first, stop=last)
                for ss in range(4):
                    o = osb.tile([128, CW], f32, tag="o", name=f"o{q}_{ss}")
                    nc.scalar.copy(o[:], o_ps[ss][:])
                    nc.sync.dma_start(
                        out[q * SQT + ss * 128:q * SQT + (ss + 1) * 128, :],
                        o[:])

    nc.compile()
    return nc


def _prep_inputs(x, wq, wk, wv, wo, freqs_cos, freqs_sin, mask):
    bf16 = ml_dtypes.bfloat16
    x2 = np.asarray(x, dtype=np.float32).reshape(S, D)
    xT = np.ascontiguousarray(x2.T).astype(bf16)
    cosT = np.repeat(np.asarray(freqs_cos, np.float32).T, 2, axis=0)
    sinT = np.repeat(np.asarray(freqs_sin, np.float32).T, 2, axis=0).copy()
    sinT[0::2] *= -1.0
    cosT = np.ascontiguousarray(cosT).astype(bf16)
    sinT = np.ascontiguousarray(sinT).astype(bf16)
    m2 = np.asarray(mask, np.float32).reshape(S, S)
    masks = np.stack([np.ascontiguousarray(m2[0:SQT, r * 128:(r + 1) * 128].T)
                      for r in range(4)]).astype(bf16)  # [4, 128, 512]
    in_maps = []
    for c in range(N_CORES):
        cols = slice(c * CW, (c + 1) * CW)
        in_maps.append({
            "xT": xT,
            "wq": np.ascontiguousarray(np.asarray(wq, np.float32)[:, cols]).astype(bf16),
            "wk": np.ascontiguousarray(np.asarray(wk, np.float32)[:, cols]).astype(bf16),
            "wv": np.ascontiguousarray(np.asarray(wv, np.float32)[:, cols]).astype(bf16),
            "wo": np.ascontiguousarray(np.asarray(wo, np.float32)[:, cols]).astype(bf16),
            "cosT": cosT,
            "ones": np.ones((HD, 1), bf16),
            "sinT": sinT,
            "masks": masks,
        })
    return in_maps


def kernel(x, wq, wk, wv, wo, freqs_cos, freqs_sin, mask):
    global LAST_RESULT
    from concourse.bass_utils import run_bass_kernel_spmd

    if "nc" not in _CACHE:
        _CACHE["nc"] = _build()
    nc = _CACHE["nc"]
    in_maps = _prep_inputs(x, wq, wk, wv, wo, freqs_cos, freqs_sin, mask)
    res = run_bass_kernel_spmd(nc, in_maps, core_ids=list(range(N_CORES)))
    LAST_RESULT = res
    out = np.concatenate([res.results[c]["out"] for c in range(N_CORES)],
                         axis=1)
    return out.reshape(B, S, D).astype(np.float32)


# revision 18
# speedup vs baseline: 1.0896x; 1.0386x over previous
"""Trainium2 Bass kernel for a LLaMA-style causal attention block.

Sharding (8 NeuronCores, one trn2 chip):
  - Tensor-parallel over heads: core c owns heads [4c, 4c+4) -> wq/wk/wv column
    slices [4096, 512]; computes qT/kT/v + RoPE + causal attention for its heads.
  - attnT [512, 2048] (bf16) is AllGather'd (chunked over 4 sq quarters, so comm
    overlaps compute) -> each core computes out[:, 512c:512c+512] = attn @ wo_cols.
  - Host concatenates the 8 column slices.

Layout trick: everything is computed transposed ([head_dim, seq]) so that no
on-device transposes are needed anywhere:
  qT/kT = w_h.T @ xT      (xT host-pretransposed)
  scoresT[sk, sq] = kT_tile.T @ qT     (softmax denom on DVE/GpSimd, not PE)
  attnT[hd, sq] = v_tile.T @ expT      (expT is exactly the scoresT layout)
  out[sq, cols] = attnT_full_tile.T @ wo_tile
RoPE is applied in the transposed layout with a DVE stream_shuffle partition
pair-swap. exp() needs no max-subtraction: scores are O(1) by construction.

Compute dtype bf16 (f32 PSUM accumulation), I/O f32.
"""

import math
import os
import sys

for _p in ("/opt/trn_rl_repo",):
    if os.path.isdir(_p) and _p not in sys.path:
        sys.path.insert(0, _p)

import numpy as np
import ml_dtypes

N_CORES = 8
B, S, D, H = 1, 2048, 4096, 32
HD = D // H          # 128
HPC = H // N_CORES   # 4 heads per core
CW = D // N_CORES    # 512 columns per core
NK = D // 128        # 32 contraction tiles
SQT = 512            # sq tile width
NSQ = S // SQT       # 4
SCALE = 1.0 / math.sqrt(HD)

_CACHE = {}
LAST_RESULT = None   # test harness reads exec_time_ns from here


def _build():
    import concourse.mybir as mybir
    import concourse.tile as tile
    from concourse import bacc, bass_isa

    dt = mybir.dt
    f32, bf16 = dt.float32, dt.bfloat16

    nc = bacc.Bacc("TRN2", target_bir_lowering=False, debug=False,
                   num_devices=N_CORES)

    xT = nc.dram_tensor("xT", [D, S], bf16, kind="ExternalInput").ap()
    wq = nc.dram_tensor("wq", [D, CW], bf16, kind="ExternalInput").ap()
    wk = nc.dram_tensor("wk", [D, CW], bf16, kind="ExternalInput").ap()
    wv = nc.dram_tensor("wv", [D, CW], bf16, kind="ExternalInput").ap()
    wo = nc.dram_tensor("wo", [D, CW], bf16, kind="ExternalInput").ap()
    cosT = nc.dram_tensor("cosT", [HD, S], bf16, kind="ExternalInput").ap()
    sinT = nc.dram_tensor("sinT", [HD, S], bf16, kind="ExternalInput").ap()
    ones = nc.dram_tensor("ones", [HD, 1], bf16, kind="ExternalInput").ap()
    masks = nc.dram_tensor("masks", [4, 128, SQT], bf16, kind="ExternalInput").ap()
    out = nc.dram_tensor("out", [S, CW], f32, kind="ExternalOutput").ap()

    swap_mask = []
    for i in range(16):
        swap_mask += [2 * i + 1, 2 * i]

    rg = [list(range(N_CORES))]

    with tile.TileContext(nc) as tc:
        with (
            tc.tile_pool(name="consts", bufs=1) as cpool,
            tc.tile_pool(name="xp", bufs=34) as xpool,
            tc.tile_pool(name="wqp", bufs=6) as wqp,
            tc.tile_pool(name="wkp", bufs=6) as wkp,
            tc.tile_pool(name="wvp", bufs=6) as wvp,
            tc.tile_pool(name="res", bufs=1) as res,
            tc.tile_pool(name="rope32", bufs=3) as rope32,
            tc.tile_pool(name="ropebf", bufs=4) as ropebf,
            tc.tile_pool(name="expp", bufs=10) as expp,
            tc.tile_pool(name="nrm", bufs=2) as nrm,
            tc.tile_pool(name="attnsb", bufs=2) as attnsb,
            tc.tile_pool(name="wop", bufs=1) as wop,
            tc.tile_pool(name="agsb", bufs=8) as agsb,
            tc.tile_pool(name="osb", bufs=2) as osb,
            tc.tile_pool(name="ps", bufs=8, space="PSUM") as ps,
            tc.tile_pool(name="dram", bufs=1, space="DRAM") as dram,
        ):
            # resident results of QKV+rope
            qrot = [res.tile([HD, S], bf16, name=f"qrot{h}") for h in range(HPC)]
            krot = [res.tile([HD, S], bf16, name=f"krot{h}") for h in range(HPC)]
            v_sb = [res.tile([128, CW], bf16, name=f"v{i}") for i in range(S // 128)]

            # AllGather bounce buffers (one per sq quarter)
            ag_in = [dram.tile([HPC * HD, SQT], bf16, name=f"agin{q}")
                     for q in range(NSQ)]
            ag_out = [dram.tile([D, SQT], bf16, addr_space="Shared",
                                name=f"agout{q}") for q in range(NSQ)]

            cos_sb = cpool.tile([HD, S], bf16, name="cos_sb")
            ones_sb = cpool.tile([HD, 1], bf16, name="ones_sb")
            sin_sb = cpool.tile([HD, S], bf16, name="sin_sb")
            mask_sb = [cpool.tile([128, SQT], bf16, name=f"mask{r}")
                       for r in range(4)]
            wo_sb = [wop.tile([128, CW], bf16, name=f"wo{d}") for d in range(NK)]

            def emit_qkv(st):
                sq0 = st * SQT
                q_ps = [ps.tile([128, SQT], f32, tag="b", name=f"qps{st}_{h}")
                        for h in range(HPC)]
                k_ps = [ps.tile([128, SQT], f32, tag="b", name=f"kps{st}_{h}")
                        for h in range(HPC)]
                x_tiles = []
                for d in range(NK):
                    xt = xpool.tile([128, SQT], bf16, tag="x", name=f"x{st}_{d}")
                    nc.sync.dma_start(xt[:], xT[d * 128:(d + 1) * 128,
                                                sq0:sq0 + SQT])
                    x_tiles.append(xt)
                    wqt = wqp.tile([128, CW], bf16, tag="wq", name=f"wq{st}_{d}")
                    nc.sync.dma_start(wqt[:], wq[d * 128:(d + 1) * 128, :])
                    wkt = wkp.tile([128, CW], bf16, tag="wk", name=f"wk{st}_{d}")
                    nc.sync.dma_start(wkt[:], wk[d * 128:(d + 1) * 128, :])
                    first, last = d == 0, d == NK - 1
                    for h in range(HPC):
                        nc.tensor.matmul(q_ps[h][:], wqt[:, h * HD:(h + 1) * HD],
                                         xt[:], start=first, stop=last)
                    for h in range(HPC):
                        nc.tensor.matmul(k_ps[h][:], wkt[:, h * HD:(h + 1) * HD],
                                         xt[:], start=first, stop=last)
                if st == 0:
                    # constants are first needed by RoPE / attention below;
                    # emitting them here keeps the first QKV DMAs in front
                    nc.sync.dma_start(cos_sb[:], cosT[:])
                    nc.sync.dma_start(sin_sb[:], sinT[:])
                    nc.sync.dma_start(ones_sb[:], ones[:])
                    for r in range(4):
                        nc.sync.dma_start(mask_sb[r][:], masks[r])
                # RoPE: rot = t*cos + shuffle(t)*sin'   (sin' sign-baked)
                for h in range(HPC):
                    for pst, rot in ((q_ps[h], qrot[h]), (k_ps[h], krot[h])):
                        tbf = ropebf.tile([128, SQT], bf16, tag="rbf",
                                          name=f"rbf{st}_{h}")
                        nc.scalar.copy(tbf[:], pst[:])
                        tsw = ropebf.tile([128, SQT], bf16, tag="rsw",
                                          name=f"rsw{st}_{h}")
                        nc.vector.stream_shuffle(tsw[:], tbf[:], swap_mask)
                        t1 = rope32.tile([128, SQT], f32, tag="r32",
                                         name=f"r1_{st}_{h}")
                        nc.vector.tensor_mul(t1[:], tbf[:],
                                             cos_sb[:, sq0:sq0 + SQT])
                        t2 = rope32.tile([128, SQT], f32, tag="r32",
                                         name=f"r2_{st}_{h}")
                        nc.vector.tensor_mul(t2[:], tsw[:],
                                             sin_sb[:, sq0:sq0 + SQT])
                        nc.vector.tensor_add(rot[:, sq0:sq0 + SQT], t1[:], t2[:])
                # V projection for this s range
                v_ps = [ps.tile([128, CW], f32, tag="b", name=f"vps{st}_{ss}")
                        for ss in range(4)]
                for d in range(NK):
                    wvt = wvp.tile([128, CW], bf16, tag="wv", name=f"wv{st}_{d}")
                    nc.sync.dma_start(wvt[:], wv[d * 128:(d + 1) * 128, :])
                    first, last = d == 0, d == NK - 1
                    for ss in range(4):
                        nc.tensor.matmul(v_ps[ss][:],
                                         x_tiles[d][:, ss * 128:(ss + 1) * 128],
                                         wvt[:], start=first, stop=last)
                for ss in range(4):
                    nc.scalar.copy(v_sb[st * 4 + ss][:], v_ps[ss][:])

            def emit_attention(sqT):
                sq0 = sqT * SQT
                nblk = 4 * (sqT + 1)
                for h in range(HPC):
                    attn_ps = ps.tile([HD, SQT], f32, tag="b",
                                      name=f"aps{sqT}_{h}")
                    den_ps = ps.tile([1, SQT], f32, tag="b",
                                     name=f"dps{sqT}_{h}")
                    exp_tiles = []

                    def emit_pv(j, h=h, attn_ps=attn_ps, den_ps=den_ps,
                                exp_tiles=exp_tiles, nblk=nblk):
                        first, last = j == 0, j == nblk - 1
                        nc.tensor.matmul(attn_ps[:],
                                         v_sb[j][:, h * HD:(h + 1) * HD],
                                         exp_tiles[j][:],
                                         start=first, stop=last)
                        nc.tensor.matmul(den_ps[:], ones_sb[:],
                                         exp_tiles[j][:],
                                         start=first, stop=last)

                    for i in range(nblk):
                        sc = ps.tile([128, SQT], f32, tag="b",
                                     name=f"sc{sqT}_{h}_{i}")
                        nc.tensor.matmul(sc[:],
                                         krot[h][:, i * 128:(i + 1) * 128],
                                         qrot[h][:, sq0:sq0 + SQT],
                                         start=True, stop=True)
                        r = i - 4 * sqT
                        if r >= 0:  # block straddles the diagonal
                            nc.vector.tensor_add(sc[:], sc[:], mask_sb[r][:])
                        e = expp.tile([128, SQT], bf16, tag="e",
                                      name=f"e{sqT}_{h}_{i}")
                        nc.scalar.activation(e[:], sc[:],
                                             mybir.ActivationFunctionType.Exp,
                                             scale=SCALE)
                        exp_tiles.append(e)
                        if i >= 2:
                            emit_pv(i - 2)
                    emit_pv(nblk - 2)
                    emit_pv(nblk - 1)

                    # evacuate PSUM right away so the next round's projections
                    # get their banks back without waiting on the normalize
                    a32 = nrm.tile([HD, SQT], f32, tag="a32",
                                   name=f"a32_{sqT}_{h}")
                    nc.scalar.copy(a32[:], attn_ps[:])
                    rec = nrm.tile([1, SQT], f32, tag="rec",
                                   name=f"rec{sqT}_{h}")
                    nc.vector.reciprocal(rec[:], den_ps[:])
                    bc = nrm.tile([128, SQT], f32, tag="bc",
                                  name=f"bc{sqT}_{h}")
                    nc.gpsimd.partition_broadcast(bc[:], rec[:], channels=128)
                    a_sb = attnsb.tile([HD, SQT], bf16, tag="a",
                                       name=f"asb{sqT}_{h}")
                    nc.vector.tensor_mul(a_sb[:], a32[:], bc[:])
                    nc.sync.dma_start(ag_in[sqT][h * HD:(h + 1) * HD, :],
                                      a_sb[:])
                # gather this sq quarter's attnT from all cores
                nc.gpsimd.collective_compute(
                    "AllGather", mybir.AluOpType.bypass, replica_groups=rg,
                    ins=[ag_in[sqT].opt()], outs=[ag_out[sqT].opt()])

            for st in range(NSQ):
                emit_qkv(st)
            for st in range(NSQ):
                emit_attention(st)
                if st == 0:
                    for d in range(NK):  # prefetch wo during attention
                        nc.sync.dma_start(wo_sb[d][:],
                                          wo[d * 128:(d + 1) * 128, :])

            # ================= output projection =================
            for q in range(NSQ):
                o_ps = [ps.tile([128, CW], f32, tag="b", name=f"ops{q}_{ss}")
                        for ss in range(4)]
                for d in range(NK):
                    agt = agsb.tile([128, SQT], bf16, tag="ag",
                                    name=f"agt{q}_{d}")
                    nc.sync.dma_start(agt[:],
                                      ag_out[q][d * 128:(d + 1) * 128, :])
                    first, last = d == 0, d == NK - 1
                    for ss in range(4):
                        nc.tensor.matmul(o_ps[ss][:],
                                         agt[:, ss * 128:(ss + 1) * 128],
                                         wo_sb[d][:], start=first, stop=last)
                for ss in range(4):
                    o = osb.tile([128, CW], f32, tag="o", name=f"o{q}_{ss}")
                    nc.scalar.copy(o[:], o_ps[ss][:])
                    nc.sync.dma_start(
                        out[q * SQT + ss * 128:q * SQT + (ss + 1) * 128, :],
                        o[:])

    nc.compile()
    return nc


def _prep_inputs(x, wq, wk, wv, wo, freqs_cos, freqs_sin, mask):
    bf16 = ml_dtypes.bfloat16
    x2 = np.asarray(x, dtype=np.float32).reshape(S, D)
    xT = np.ascontiguousarray(x2.T).astype(bf16)
    cosT = np.repeat(np.asarray(freqs_cos, np.float32).T, 2, axis=0)
    sinT = np.repeat(np.asarray(freqs_sin, np.float32).T, 2, axis=0).copy()
    sinT[0::2] *= -1.0
    cosT = np.ascontiguousarray(cosT).astype(bf16)
    sinT = np.ascontiguousarray(sinT).astype(bf16)
    m2 = np.asarray(mask, np.float32).reshape(S, S)
    masks = np.stack([np.ascontiguousarray(m2[0:SQT, r * 128:(r + 1) * 128].T)
                      for r in range(4)]).astype(bf16)  # [4, 128, 512]
    in_maps = []
    for c in range(N_CORES):
        cols = slice(c * CW, (c + 1) * CW)
        in_maps.append({
            "xT": xT,
            "wq": np.ascontiguousarray(np.asarray(wq, np.float32)[:, cols]).astype(bf16),
            "wk": np.ascontiguousarray(np.asarray(wk, np.float32)[:, cols]).astype(bf16),
            "wv": np.ascontiguousarray(np.asarray(wv, np.float32)[:, cols]).astype(bf16),
            "wo": np.ascontiguousarray(np.asarray(wo, np.float32)[:, cols]).astype(bf16),
            "cosT": cosT,
            "ones": np.ones((HD, 1), bf16),
            "sinT": sinT,
            "masks": masks,
        })
    return in_maps


def kernel(x, wq, wk, wv, wo, freqs_cos, freqs_sin, mask):
    global LAST_RESULT
    from concourse.bass_utils import run_bass_kernel_spmd

    if "nc" not in _CACHE:
        _CACHE["nc"] = _build()
    nc = _CACHE["nc"]
    in_maps = _prep_inputs(x, wq, wk, wv, wo, freqs_cos, freqs_sin, mask)
    res = run_bass_kernel_spmd(nc, in_maps, core_ids=list(range(N_CORES)))
    LAST_RESULT = res
    out = np.concatenate([res.results[c]["out"] for c in range(N_CORES)],
                         axis=1)
    return out.reshape(B, S, D).astype(np.float32)
